# revision 16
# baseline (speedup 1.0000x reference)
"""Trainium2 Bass kernel for nn_ADDLossSoftEncode (Davenport q-method ADD loss).

Data parallel over batch: B=512 sharded as 64 samples per core across 8 cores.
Per core:
  stage A: e = exp(s - max), Gram A_ij = sum_k e_k q_ki q_kj via fused
           tensor_tensor_reduce ops (layout: partition = khalf*64 + sample).
  stage B: power iteration on 4x4 A per sample -> top eigenvector q_pred;
           rotation matrices R(q_pred), R(gt); G = dR dR^T (3x3).
  stage C: per-point squared norm p^T G p via Horner form, sqrt+accumulate
           on ACT -> per-partition partial sums.
Host: sum partials over cores/partitions, divide by B*P.
"""

import sys
from contextlib import ExitStack

import numpy as np

sys.path.insert(0, "/opt/trn_rl_repo")

import concourse.bass as bass
import concourse.tile as tile
from concourse import bacc
from concourse import mybir

F32 = mybir.dt.float32
AX = mybir.AxisListType
OP = mybir.AluOpType
ACT = mybir.ActivationFunctionType

B, K, P = 512, 8192, 4096
NCORES = 8
S = B // NCORES          # 64 samples per core
KH = K // 2              # 4096 per k-half
PH = P // 2              # 2048 per point-half
KC = 1024                # k-chunk (per half)
PC = 1024                # point chunk (per half)
NKC = KH // KC           # 4
NPC = PH // PC           # 2

# unique Gram entries: 9 computed pairs + A33 via trace identity
PAIRS = [(0, 0), (0, 1), (0, 2), (0, 3), (1, 1), (1, 2), (1, 3), (2, 2), (2, 3)]
UIDX = {p: n for n, p in enumerate(PAIRS)}
UIDX[(3, 3)] = 9
NSQ = 13  # matrix squarings (A -> A^(2^NSQ))


def _emit(ctx, tc, sep, ori, gt, pt, out):
    nc = tc.nc
    pool_big = ctx.enter_context(tc.tile_pool(name="big", bufs=2))
    pool_q = ctx.enter_context(tc.tile_pool(name="q", bufs=2))
    pool_u = ctx.enter_context(tc.tile_pool(name="u", bufs=2))
    pool_scr = ctx.enter_context(tc.tile_pool(name="scr", bufs=2))
    pool_st = ctx.enter_context(tc.tile_pool(name="st", bufs=1))
    pool_tiny = ctx.enter_context(tc.tile_pool(name="tiny", bufs=1))
    pool_pt = ctx.enter_context(tc.tile_pool(name="pt", bufs=2))
    pool_c = ctx.enter_context(tc.tile_pool(name="c", bufs=2))

    # ---------------- stage A: softmax weights + Gram matrix ----------------
    s_sb = pool_st.tile([128, KH], F32)
    nc.sync.dma_start(s_sb[0:S, :], sep[:, 0:KH])
    nc.sync.dma_start(s_sb[S:128, :], sep[:, KH:K])

    m2 = pool_tiny.tile([128, 1], F32)
    nc.vector.tensor_reduce(m2[:], s_sb[:], axis=AX.X, op=OP.max)
    negm = pool_tiny.tile([128, 1], F32)
    mup = pool_tiny.tile([64, 1], F32)
    # combined max over both k-halves of each sample, negated, broadcast back
    nc.sync.dma_start(mup[:], m2[S:128, :])
    nc.vector.tensor_tensor(negm[0:S, :], m2[0:S, :], mup[:], op=OP.max)
    nc.scalar.mul(negm[0:S, :], negm[0:S, :], -1.0)
    nc.sync.dma_start(negm[S:128, :], negm[0:S, :])

    e_sb = pool_st.tile([128, KH], F32)
    sumE = pool_tiny.tile([128, 1], F32)
    nc.scalar.activation(e_sb[:], s_sb[:], ACT.Exp, bias=negm[:], scale=1.0,
                         accum_out=sumE[:])

    # Gram accumulators per chunk
    aacc = pool_tiny.tile([128, 9 * NKC], F32)
    for c in range(NKC):
        qc = pool_q.tile([128, KC * 4], F32)
        nc.sync.dma_start(qc[0:S, :],
                            ori[:, c * KC:(c + 1) * KC, :].rearrange("s k q -> s (k q)"))
        nc.sync.dma_start(qc[S:128, :],
                            ori[:, KH + c * KC:KH + (c + 1) * KC, :].rearrange("s k q -> s (k q)"))
        ec = e_sb[:, c * KC:(c + 1) * KC]
        us = []
        for i in range(3):
            ui = pool_u.tile([128, KC], F32, tag=f"u{i}")
            nc.vector.tensor_tensor(ui[:], ec, qc[:, i::4], op=OP.mult)
            us.append(ui)
        for n, (i, j) in enumerate(PAIRS):
            scr = pool_scr.tile([128, KC], F32)
            nc.vector.tensor_tensor(scr[:], us[i][:], qc[:, j::4], op=OP.mult)
            scr2 = pool_scr.tile([128, KC], F32, tag="scr2")
            nc.scalar.activation(scr2[:], scr[:], ACT.Identity,
                                 accum_out=aacc[:, n * NKC + c:n * NKC + c + 1])

    # sum chunks -> [128, 9]; then halves; then A33 from trace identity
    a128 = pool_tiny.tile([128, 9], F32)
    nc.vector.tensor_reduce(a128[:], aacc[:].rearrange("p (n c) -> p n c", c=NKC), axis=AX.X,
                            op=OP.add)
    a64 = pool_tiny.tile([64, 10], F32)
    aup = pool_tiny.tile([64, 9], F32)
    nc.sync.dma_start(aup[:], a128[S:128, :])
    nc.vector.tensor_tensor(a64[:, 0:9], a128[0:S, :], aup[:], op=OP.add)
    sE = pool_tiny.tile([64, 1], F32)
    sup = pool_tiny.tile([64, 1], F32)
    nc.sync.dma_start(sup[:], sumE[S:128, :])
    nc.vector.tensor_tensor(sE[:], sumE[0:S, :], sup[:], op=OP.add)
    nc.vector.tensor_tensor(a64[:, 9:10], sE[:], a64[:, UIDX[(0, 0)]:UIDX[(0, 0)] + 1],
                            op=OP.subtract)
    nc.vector.tensor_tensor(a64[:, 9:10], a64[:, 9:10],
                            a64[:, UIDX[(1, 1)]:UIDX[(1, 1)] + 1], op=OP.subtract)
    nc.vector.tensor_tensor(a64[:, 9:10], a64[:, 9:10],
                            a64[:, UIDX[(2, 2)]:UIDX[(2, 2)] + 1], op=OP.subtract)

    # full 4x4 matrix [64, 16] row-major (i*4+j)
    amat = pool_tiny.tile([64, 16], F32)
    for i in range(4):
        for j in range(4):
            u = UIDX[(min(i, j), max(i, j))]
            nc.vector.tensor_copy(amat[:, i * 4 + j:i * 4 + j + 1], a64[:, u:u + 1])

    # ---------------- stage B: matrix squaring (A -> A^8192) ----------------
    # eigengap is ~1.002-1.03, so plain power iteration cannot converge;
    # 13 trace-normalized squarings give (l1/l2)^8192 separation.
    a_cur = amat
    trv = pool_tiny.tile([64, 1], F32)
    tri = pool_tiny.tile([64, 1], F32)
    for m in range(NSQ):
        a_new = pool_tiny.tile([64, 16], F32, tag=f"asq{m % 2}")
        t16 = pool_tiny.tile([64, 16], F32, tag="t16")
        s3 = a_cur[:].rearrange("p (i j) -> p i j", j=4)
        d3 = a_new[:].rearrange("p (i j) -> p i j", j=4)
        t3 = t16[:].rearrange("p (i j) -> p i j", j=4)
        for j in range(4):
            colj = s3[:, :, j].unsqueeze(2).broadcast_to([64, 4, 4])
            rowj = s3[:, j, :].unsqueeze(1).broadcast_to([64, 4, 4])
            if j == 0:
                nc.vector.tensor_tensor(d3, colj, rowj, op=OP.mult)
            else:
                nc.vector.tensor_tensor(t3, colj, rowj, op=OP.mult)
                nc.vector.tensor_tensor(d3, d3, t3, op=OP.add)
        nc.vector.tensor_reduce(trv[:], a_new[:, 0::5], axis=AX.X, op=OP.add)
        nc.vector.reciprocal(tri[:], trv[:])
        nc.vector.tensor_scalar(a_new[:], a_new[:], tri[:], None, op0=OP.mult)
        a_cur = a_new

    # v = A^N @ x0 for fixed x0
    v = pool_tiny.tile([64, 4], F32)
    wt = pool_tiny.tile([64, 4], F32)
    x0 = [1.0, 0.61, 0.37, 0.22]
    for j in range(4):
        colj = a_cur[:, j::4]
        if j == 0:
            nc.scalar.mul(v[:], colj, x0[j])
        else:
            nc.scalar.mul(wt[:], colj, x0[j])
            nc.vector.tensor_tensor(v[:], v[:], wt[:], op=OP.add)

    gtt = pool_tiny.tile([64, 4], F32)
    nc.sync.dma_start(gtt[:], gt[:, :])

    def quat_to_R(q, Rt):
        # outer products o[a*4+b] = q_a*q_b  via 4 tensor_scalar ops
        o = pool_tiny.tile([64, 16], F32, tag="outer")
        for a in range(4):
            nc.vector.tensor_scalar(o[:, a * 4:a * 4 + 4], q[:], q[:, a:a + 1],
                                    None, op0=OP.mult)
        nsq = pool_tiny.tile([64, 1], F32, tag="nsq")
        nc.vector.tensor_tensor(nsq[:], o[:, 0:1], o[:, 5:6], op=OP.add)
        nc.vector.tensor_tensor(nsq[:], nsq[:], o[:, 10:11], op=OP.add)
        nc.vector.tensor_tensor(nsq[:], nsq[:], o[:, 15:16], op=OP.add)
        ts2 = pool_tiny.tile([64, 1], F32, tag="ts2")
        nc.vector.reciprocal(ts2[:], nsq[:])
        nc.scalar.mul(ts2[:], ts2[:], 2.0)
        nts = pool_tiny.tile([64, 1], F32, tag="nts")
        nc.scalar.mul(nts[:], ts2[:], -1.0)
        t = pool_tiny.tile([64, 1], F32, tag="qrt")
        # (r,i,j,k) = (0,1,2,3); o[a,b] at col a*4+b
        def oc(a, b):
            return o[:, a * 4 + b:a * 4 + b + 1]
        # diag entries: R_dd = 1 - ts*(x+y)
        for d, (u1, u2) in enumerate([((2, 2), (3, 3)), ((1, 1), (3, 3)),
                                      ((1, 1), (2, 2))]):
            nc.vector.tensor_tensor(t[:], oc(*u1), oc(*u2), op=OP.add)
            nc.vector.tensor_scalar(Rt[:, d * 3 + d:d * 3 + d + 1], t[:], nts[:],
                                    1.0, op0=OP.mult, op1=OP.add)
        # off-diag: R_mn = ts*(o1 +/- o2)
        offd = [(0, 1, (1, 2), (3, 0), OP.subtract), (0, 2, (1, 3), (2, 0), OP.add),
                (1, 0, (1, 2), (3, 0), OP.add), (1, 2, (2, 3), (1, 0), OP.subtract),
                (2, 0, (1, 3), (2, 0), OP.subtract), (2, 1, (2, 3), (1, 0), OP.add)]
        for m, n, p1, p2, op in offd:
            nc.vector.tensor_tensor(t[:], oc(*p1), oc(*p2), op=op)
            nc.vector.tensor_scalar(Rt[:, m * 3 + n:m * 3 + n + 1], t[:], ts2[:],
                                    None, op0=OP.mult)

    Rp = pool_tiny.tile([64, 9], F32)
    Rg = pool_tiny.tile([64, 9], F32)
    quat_to_R(v, Rp)
    quat_to_R(gtt, Rg)
    dR = pool_tiny.tile([64, 9], F32)
    nc.vector.tensor_tensor(dR[:], Rp[:], Rg[:], op=OP.subtract)

    # G (3x3 sym of dR dR^T): cols [G00,G11,G22,2G01,2G02,2G12] on 128 partitions
    gp = pool_tiny.tile([128, 6], F32)
    gscr = pool_tiny.tile([64, 3], F32)
    gpairs = [(0, 0, 1.0), (1, 1, 1.0), (2, 2, 1.0), (0, 1, 2.0), (0, 2, 2.0),
              (1, 2, 2.0)]
    for n, (a, b, sc) in enumerate(gpairs):
        nc.vector.tensor_tensor(gscr[:], dR[:, 3 * a:3 * a + 3],
                                dR[:, 3 * b:3 * b + 3], op=OP.mult)
        nc.vector.tensor_reduce(gp[0:S, n:n + 1], gscr[:], axis=AX.X, op=OP.add)
        if sc != 1.0:
            nc.scalar.mul(gp[0:S, n:n + 1], gp[0:S, n:n + 1], sc)
    nc.sync.dma_start(gp[S:128, :], gp[0:S, :])

    # ---------------- stage C: per-point norms ----------------
    pn = pool_tiny.tile([128, NPC], F32)
    sqb = pool_tiny.tile([128, 1], F32)
    nc.vector.memset(sqb[:], 1e-5)
    for c in range(NPC):
        ptc = pool_pt.tile([128, PC * 3], F32)
        nc.sync.dma_start(ptc[0:S, :],
                            pt[:, c * PC:(c + 1) * PC, :].rearrange("s k q -> s (k q)"))
        nc.sync.dma_start(ptc[S:128, :],
                            pt[:, PH + c * PC:PH + (c + 1) * PC, :].rearrange("s k q -> s (k q)"))
        x, y, z = ptc[:, 0::3], ptc[:, 1::3], ptc[:, 2::3]

        def g(n):
            return gp[:, n:n + 1]
        A1 = pool_c.tile([128, PC], F32, tag="A1")
        t1 = pool_c.tile([128, PC], F32, tag="t1")
        nc.vector.tensor_scalar(A1[:], x, g(0), None, op0=OP.mult)
        nc.vector.tensor_scalar(t1[:], y, g(3), None, op0=OP.mult)
        nc.vector.tensor_tensor(A1[:], A1[:], t1[:], op=OP.add)
        nc.vector.tensor_scalar(t1[:], z, g(4), None, op0=OP.mult)
        nc.vector.tensor_tensor(A1[:], A1[:], t1[:], op=OP.add)
        B1 = pool_c.tile([128, PC], F32, tag="B1")
        nc.vector.tensor_scalar(B1[:], y, g(1), None, op0=OP.mult)
        nc.vector.tensor_scalar(t1[:], z, g(5), None, op0=OP.mult)
        nc.vector.tensor_tensor(B1[:], B1[:], t1[:], op=OP.add)
        C1 = pool_c.tile([128, PC], F32, tag="C1")
        nc.vector.tensor_scalar(C1[:], z, g(2), None, op0=OP.mult)
        # S = x*A1 + y*B1 + z*C1
        nc.vector.tensor_tensor(A1[:], A1[:], x, op=OP.mult)
        nc.vector.tensor_tensor(B1[:], B1[:], y, op=OP.mult)
        nc.vector.tensor_tensor(C1[:], C1[:], z, op=OP.mult)
        nc.vector.tensor_tensor(A1[:], A1[:], B1[:], op=OP.add)
        nc.vector.tensor_tensor(A1[:], A1[:], C1[:], op=OP.add)
        sq = pool_c.tile([128, PC], F32, tag="sq")
        nc.scalar.activation(sq[:], A1[:], ACT.Sqrt, bias=sqb[:], scale=1.0,
                             accum_out=pn[:, c:c + 1])

    partial = pool_tiny.tile([128, 1], F32)
    nc.vector.tensor_reduce(partial[:], pn[:], axis=AX.X, op=OP.add)
    nc.sync.dma_start(out[:, :], partial[:])


_NC_CACHE = {}


def _build():
    if "nc" in _NC_CACHE:
        return _NC_CACHE["nc"]
    nc = bacc.Bacc("TRN2", target_bir_lowering=False, debug=False, enable_asserts=True, num_devices=NCORES)
    sep = nc.declare_dram_parameter("softEncodePred", [S, K], F32, isOutput=False)
    ori = nc.declare_dram_parameter("oriHistogramMap", [S, K, 4], F32, isOutput=False)
    gt = nc.declare_dram_parameter("gt", [S, 4], F32, isOutput=False)
    pt = nc.declare_dram_parameter("point", [S, P, 3], F32, isOutput=False)
    out = nc.declare_dram_parameter("out", [128, 1], F32, isOutput=True)
    with tile.TileContext(nc) as tc:
        with ExitStack() as ctx:
            _emit(ctx, tc, sep.ap(), ori.ap(), gt.ap(), pt.ap(), out.ap())
    nc.finalize()
    _NC_CACHE["nc"] = nc
    return nc


def kernel(softEncodePred, oriHistogramMap, gt, point):
    nc = _build()
    in_maps = []
    for c in range(NCORES):
        sl = slice(c * S, (c + 1) * S)
        in_maps.append({
            "softEncodePred": np.ascontiguousarray(softEncodePred[sl], np.float32),
            "oriHistogramMap": np.ascontiguousarray(oriHistogramMap[sl], np.float32),
            "gt": np.ascontiguousarray(gt[sl], np.float32),
            "point": np.ascontiguousarray(point[sl], np.float32),
        })
    from concourse.bass_utils import run_bass_kernel_spmd
    res = run_bass_kernel_spmd(nc, in_maps, core_ids=list(range(NCORES)))
    total = np.float64(0.0)
    for r in res.results:
        total += np.asarray(r["out"], np.float64).sum()
    return np.float32(total / (B * P))


# revision 17
# speedup vs baseline: 1.2020x; 1.2020x over previous
"""Trainium2 Bass kernel for nn_ADDLossSoftEncode (Davenport q-method ADD loss).

Data parallel over batch: B=512 sharded as 64 samples per core across 8 cores.
Per core:
  stage A: e = exp(s - max), Gram A_ij = sum_k e_k q_ki q_kj via fused
           tensor_tensor_reduce ops (layout: partition = khalf*64 + sample).
  stage B: power iteration on 4x4 A per sample -> top eigenvector q_pred;
           rotation matrices R(q_pred), R(gt); G = dR dR^T (3x3).
  stage C: per-point squared norm p^T G p via Horner form, sqrt+accumulate
           on ACT -> per-partition partial sums.
Host: sum partials over cores/partitions, divide by B*P.
"""

import sys
from contextlib import ExitStack

import numpy as np

sys.path.insert(0, "/opt/trn_rl_repo")

import concourse.bass as bass
import concourse.tile as tile
from concourse import bacc
from concourse import mybir

F32 = mybir.dt.float32
AX = mybir.AxisListType
OP = mybir.AluOpType
ACT = mybir.ActivationFunctionType

B, K, P = 512, 8192, 4096
NCORES = 8
S = B // NCORES          # 64 samples per core
KH = K // 2              # 4096 per k-half
PH = P // 2              # 2048 per point-half
KC = 1024                # k-chunk (per half)
PC = 1024                # point chunk (per half)
NKC = KH // KC           # 4
NPC = PH // PC           # 2

# unique Gram entries: 9 computed pairs + A33 via trace identity
PAIRS = [(0, 0), (0, 1), (0, 2), (0, 3), (1, 1), (1, 2), (1, 3), (2, 2), (2, 3)]
UIDX = {p: n for n, p in enumerate(PAIRS)}
UIDX[(3, 3)] = 9
NSQ = 13  # matrix squarings (A -> A^(2^NSQ))


def _emit(ctx, tc, sep, ori, gt, pt, out):
    nc = tc.nc
    pool_big = ctx.enter_context(tc.tile_pool(name="big", bufs=2))
    pool_q = ctx.enter_context(tc.tile_pool(name="q", bufs=2))
    pool_u = ctx.enter_context(tc.tile_pool(name="u", bufs=2))
    pool_scr = ctx.enter_context(tc.tile_pool(name="scr", bufs=2))
    pool_st = ctx.enter_context(tc.tile_pool(name="st", bufs=1))
    pool_tiny = ctx.enter_context(tc.tile_pool(name="tiny", bufs=1))
    pool_pt = ctx.enter_context(tc.tile_pool(name="pt", bufs=2))
    pool_c = ctx.enter_context(tc.tile_pool(name="c", bufs=2))

    # ---------------- stage A: softmax weights + Gram matrix ----------------
    s_sb = pool_st.tile([128, KH], F32)
    nc.sync.dma_start(s_sb[0:S, :], sep[:, 0:KH])
    nc.sync.dma_start(s_sb[S:128, :], sep[:, KH:K])

    m2 = pool_tiny.tile([128, 1], F32)
    nc.vector.tensor_reduce(m2[:], s_sb[:], axis=AX.X, op=OP.max)
    negm = pool_tiny.tile([128, 1], F32)
    mup = pool_tiny.tile([64, 1], F32)
    # combined max over both k-halves of each sample, negated, broadcast back
    nc.sync.dma_start(mup[:], m2[S:128, :])
    nc.vector.tensor_tensor(negm[0:S, :], m2[0:S, :], mup[:], op=OP.max)
    nc.scalar.mul(negm[0:S, :], negm[0:S, :], -1.0)
    nc.sync.dma_start(negm[S:128, :], negm[0:S, :])

    e_sb = pool_st.tile([128, KH], F32)
    sumE = pool_tiny.tile([128, 1], F32)
    nc.scalar.activation(e_sb[:], s_sb[:], ACT.Exp, bias=negm[:], scale=1.0,
                         accum_out=sumE[:])

    # Gram accumulators per chunk
    aacc = pool_tiny.tile([128, 9 * NKC], F32)
    for c in range(NKC):
        qc = pool_q.tile([128, KC * 4], F32)
        nc.sync.dma_start(qc[0:S, :],
                            ori[:, c * KC:(c + 1) * KC, :].rearrange("s k q -> s (k q)"))
        nc.sync.dma_start(qc[S:128, :],
                            ori[:, KH + c * KC:KH + (c + 1) * KC, :].rearrange("s k q -> s (k q)"))
        ec = e_sb[:, c * KC:(c + 1) * KC]
        us = []
        for i in range(3):
            ui = pool_u.tile([128, KC], F32, tag=f"u{i}")
            nc.gpsimd.tensor_tensor(ui[:], ec, qc[:, i::4], op=OP.mult)
            us.append(ui)
        for n, (i, j) in enumerate(PAIRS):
            scr = pool_scr.tile([128, KC], F32)
            eng = nc.gpsimd if n % 5 == 4 else nc.vector
            eng.tensor_tensor(scr[:], us[i][:], qc[:, j::4], op=OP.mult)
            scr2 = pool_scr.tile([128, KC], F32, tag="scr2")
            nc.scalar.activation(scr2[:], scr[:], ACT.Identity,
                                 accum_out=aacc[:, n * NKC + c:n * NKC + c + 1])

    # sum chunks -> [128, 9]; then halves; then A33 from trace identity
    a128 = pool_tiny.tile([128, 9], F32)
    nc.vector.tensor_reduce(a128[:], aacc[:].rearrange("p (n c) -> p n c", c=NKC), axis=AX.X,
                            op=OP.add)
    a64 = pool_tiny.tile([64, 10], F32)
    aup = pool_tiny.tile([64, 9], F32)
    nc.sync.dma_start(aup[:], a128[S:128, :])
    nc.vector.tensor_tensor(a64[:, 0:9], a128[0:S, :], aup[:], op=OP.add)
    sE = pool_tiny.tile([64, 1], F32)
    sup = pool_tiny.tile([64, 1], F32)
    nc.sync.dma_start(sup[:], sumE[S:128, :])
    nc.vector.tensor_tensor(sE[:], sumE[0:S, :], sup[:], op=OP.add)
    nc.vector.tensor_tensor(a64[:, 9:10], sE[:], a64[:, UIDX[(0, 0)]:UIDX[(0, 0)] + 1],
                            op=OP.subtract)
    nc.vector.tensor_tensor(a64[:, 9:10], a64[:, 9:10],
                            a64[:, UIDX[(1, 1)]:UIDX[(1, 1)] + 1], op=OP.subtract)
    nc.vector.tensor_tensor(a64[:, 9:10], a64[:, 9:10],
                            a64[:, UIDX[(2, 2)]:UIDX[(2, 2)] + 1], op=OP.subtract)

    # full 4x4 matrix [64, 16] row-major (i*4+j)
    amat = pool_tiny.tile([64, 16], F32)
    for i in range(4):
        for j in range(4):
            u = UIDX[(min(i, j), max(i, j))]
            nc.vector.tensor_copy(amat[:, i * 4 + j:i * 4 + j + 1], a64[:, u:u + 1])

    # ---------------- stage B: matrix squaring (A -> A^8192) ----------------
    # eigengap is ~1.002-1.03, so plain power iteration cannot converge;
    # 13 trace-normalized squarings give (l1/l2)^8192 separation.
    a_cur = amat
    trv = pool_tiny.tile([64, 1], F32)
    tri = pool_tiny.tile([64, 1], F32)
    for m in range(NSQ):
        a_new = pool_tiny.tile([64, 16], F32, tag=f"asq{m % 2}")
        t16 = pool_tiny.tile([64, 16], F32, tag="t16")
        s3 = a_cur[:].rearrange("p (i j) -> p i j", j=4)
        d3 = a_new[:].rearrange("p (i j) -> p i j", j=4)
        t3 = t16[:].rearrange("p (i j) -> p i j", j=4)
        for j in range(4):
            colj = s3[:, :, j].unsqueeze(2).broadcast_to([64, 4, 4])
            rowj = s3[:, j, :].unsqueeze(1).broadcast_to([64, 4, 4])
            if j == 0:
                nc.vector.tensor_tensor(d3, colj, rowj, op=OP.mult)
            else:
                nc.vector.tensor_tensor(t3, colj, rowj, op=OP.mult)
                nc.vector.tensor_tensor(d3, d3, t3, op=OP.add)
        nc.vector.tensor_reduce(trv[:], a_new[:, 0::5], axis=AX.X, op=OP.add)
        nc.vector.reciprocal(tri[:], trv[:])
        nc.vector.tensor_scalar(a_new[:], a_new[:], tri[:], None, op0=OP.mult)
        a_cur = a_new

    # v = A^N @ x0 for fixed x0
    v = pool_tiny.tile([64, 4], F32)
    wt = pool_tiny.tile([64, 4], F32)
    x0 = [1.0, 0.61, 0.37, 0.22]
    for j in range(4):
        colj = a_cur[:, j::4]
        if j == 0:
            nc.scalar.mul(v[:], colj, x0[j])
        else:
            nc.scalar.mul(wt[:], colj, x0[j])
            nc.vector.tensor_tensor(v[:], v[:], wt[:], op=OP.add)

    gtt = pool_tiny.tile([64, 4], F32)
    nc.sync.dma_start(gtt[:], gt[:, :])

    def quat_to_R(q, Rt):
        # outer products o[a*4+b] = q_a*q_b  via 4 tensor_scalar ops
        o = pool_tiny.tile([64, 16], F32, tag="outer")
        for a in range(4):
            nc.vector.tensor_scalar(o[:, a * 4:a * 4 + 4], q[:], q[:, a:a + 1],
                                    None, op0=OP.mult)
        nsq = pool_tiny.tile([64, 1], F32, tag="nsq")
        nc.vector.tensor_tensor(nsq[:], o[:, 0:1], o[:, 5:6], op=OP.add)
        nc.vector.tensor_tensor(nsq[:], nsq[:], o[:, 10:11], op=OP.add)
        nc.vector.tensor_tensor(nsq[:], nsq[:], o[:, 15:16], op=OP.add)
        ts2 = pool_tiny.tile([64, 1], F32, tag="ts2")
        nc.vector.reciprocal(ts2[:], nsq[:])
        nc.scalar.mul(ts2[:], ts2[:], 2.0)
        nts = pool_tiny.tile([64, 1], F32, tag="nts")
        nc.scalar.mul(nts[:], ts2[:], -1.0)
        t = pool_tiny.tile([64, 1], F32, tag="qrt")
        # (r,i,j,k) = (0,1,2,3); o[a,b] at col a*4+b
        def oc(a, b):
            return o[:, a * 4 + b:a * 4 + b + 1]
        # diag entries: R_dd = 1 - ts*(x+y)
        for d, (u1, u2) in enumerate([((2, 2), (3, 3)), ((1, 1), (3, 3)),
                                      ((1, 1), (2, 2))]):
            nc.vector.tensor_tensor(t[:], oc(*u1), oc(*u2), op=OP.add)
            nc.vector.tensor_scalar(Rt[:, d * 3 + d:d * 3 + d + 1], t[:], nts[:],
                                    1.0, op0=OP.mult, op1=OP.add)
        # off-diag: R_mn = ts*(o1 +/- o2)
        offd = [(0, 1, (1, 2), (3, 0), OP.subtract), (0, 2, (1, 3), (2, 0), OP.add),
                (1, 0, (1, 2), (3, 0), OP.add), (1, 2, (2, 3), (1, 0), OP.subtract),
                (2, 0, (1, 3), (2, 0), OP.subtract), (2, 1, (2, 3), (1, 0), OP.add)]
        for m, n, p1, p2, op in offd:
            nc.vector.tensor_tensor(t[:], oc(*p1), oc(*p2), op=op)
            nc.vector.tensor_scalar(Rt[:, m * 3 + n:m * 3 + n + 1], t[:], ts2[:],
                                    None, op0=OP.mult)

    Rp = pool_tiny.tile([64, 9], F32)
    Rg = pool_tiny.tile([64, 9], F32)
    quat_to_R(v, Rp)
    quat_to_R(gtt, Rg)
    dR = pool_tiny.tile([64, 9], F32)
    nc.vector.tensor_tensor(dR[:], Rp[:], Rg[:], op=OP.subtract)

    # G (3x3 sym of dR dR^T): cols [G00,G11,G22,2G01,2G02,2G12] on 128 partitions
    gp = pool_tiny.tile([128, 6], F32)
    gscr = pool_tiny.tile([64, 3], F32)
    gpairs = [(0, 0, 1.0), (1, 1, 1.0), (2, 2, 1.0), (0, 1, 2.0), (0, 2, 2.0),
              (1, 2, 2.0)]
    for n, (a, b, sc) in enumerate(gpairs):
        nc.vector.tensor_tensor(gscr[:], dR[:, 3 * a:3 * a + 3],
                                dR[:, 3 * b:3 * b + 3], op=OP.mult)
        nc.vector.tensor_reduce(gp[0:S, n:n + 1], gscr[:], axis=AX.X, op=OP.add)
        if sc != 1.0:
            nc.scalar.mul(gp[0:S, n:n + 1], gp[0:S, n:n + 1], sc)
    nc.sync.dma_start(gp[S:128, :], gp[0:S, :])

    # ---------------- stage C: per-point norms ----------------
    pn = pool_tiny.tile([128, NPC], F32)
    sqb = pool_tiny.tile([128, 1], F32)
    nc.vector.memset(sqb[:], 1e-5)
    for c in range(NPC):
        ptc = pool_pt.tile([128, PC * 3], F32)
        nc.sync.dma_start(ptc[0:S, :],
                            pt[:, c * PC:(c + 1) * PC, :].rearrange("s k q -> s (k q)"))
        nc.sync.dma_start(ptc[S:128, :],
                            pt[:, PH + c * PC:PH + (c + 1) * PC, :].rearrange("s k q -> s (k q)"))
        x, y, z = ptc[:, 0::3], ptc[:, 1::3], ptc[:, 2::3]

        def g(n):
            return gp[:, n:n + 1]
        A1 = pool_c.tile([128, PC], F32, tag="A1")
        t1 = pool_c.tile([128, PC], F32, tag="t1")
        nc.scalar.activation(A1[:], x, ACT.Identity, scale=g(0))
        nc.scalar.activation(t1[:], y, ACT.Identity, scale=g(3))
        nc.vector.tensor_tensor(A1[:], A1[:], t1[:], op=OP.add)
        t1b = pool_c.tile([128, PC], F32, tag="t1b")
        nc.scalar.activation(t1b[:], z, ACT.Identity, scale=g(4))
        nc.vector.tensor_tensor(A1[:], A1[:], t1b[:], op=OP.add)
        B1 = pool_c.tile([128, PC], F32, tag="B1")
        nc.scalar.activation(B1[:], y, ACT.Identity, scale=g(1))
        t1c = pool_c.tile([128, PC], F32, tag="t1c")
        nc.scalar.activation(t1c[:], z, ACT.Identity, scale=g(5))
        nc.vector.tensor_tensor(B1[:], B1[:], t1c[:], op=OP.add)
        C1 = pool_c.tile([128, PC], F32, tag="C1")
        nc.scalar.activation(C1[:], z, ACT.Identity, scale=g(2))
        # S = x*A1 + y*B1 + z*C1
        nc.gpsimd.tensor_tensor(A1[:], A1[:], x, op=OP.mult)
        nc.gpsimd.tensor_tensor(B1[:], B1[:], y, op=OP.mult)
        nc.vector.tensor_tensor(C1[:], C1[:], z, op=OP.mult)
        nc.vector.tensor_tensor(A1[:], A1[:], B1[:], op=OP.add)
        nc.vector.tensor_tensor(A1[:], A1[:], C1[:], op=OP.add)
        sq = pool_c.tile([128, PC], F32, tag="sq")
        nc.scalar.activation(sq[:], A1[:], ACT.Sqrt, bias=sqb[:], scale=1.0,
                             accum_out=pn[:, c:c + 1])

    partial = pool_tiny.tile([128, 1], F32)
    nc.vector.tensor_reduce(partial[:], pn[:], axis=AX.X, op=OP.add)
    nc.sync.dma_start(out[:, :], partial[:])


_NC_CACHE = {}


def _build():
    if "nc" in _NC_CACHE:
        return _NC_CACHE["nc"]
    nc = bacc.Bacc("TRN2", target_bir_lowering=False, debug=False, enable_asserts=True, num_devices=NCORES)
    sep = nc.declare_dram_parameter("softEncodePred", [S, K], F32, isOutput=False)
    ori = nc.declare_dram_parameter("oriHistogramMap", [S, K, 4], F32, isOutput=False)
    gt = nc.declare_dram_parameter("gt", [S, 4], F32, isOutput=False)
    pt = nc.declare_dram_parameter("point", [S, P, 3], F32, isOutput=False)
    out = nc.declare_dram_parameter("out", [128, 1], F32, isOutput=True)
    with tile.TileContext(nc) as tc:
        with ExitStack() as ctx:
            _emit(ctx, tc, sep.ap(), ori.ap(), gt.ap(), pt.ap(), out.ap())
    nc.finalize()
    _NC_CACHE["nc"] = nc
    return nc


def kernel(softEncodePred, oriHistogramMap, gt, point):
    nc = _build()
    in_maps = []
    for c in range(NCORES):
        sl = slice(c * S, (c + 1) * S)
        in_maps.append({
            "softEncodePred": np.ascontiguousarray(softEncodePred[sl], np.float32),
            "oriHistogramMap": np.ascontiguousarray(oriHistogramMap[sl], np.float32),
            "gt": np.ascontiguousarray(gt[sl], np.float32),
            "point": np.ascontiguousarray(point[sl], np.float32),
        })
    from concourse.bass_utils import run_bass_kernel_spmd
    res = run_bass_kernel_spmd(nc, in_maps, core_ids=list(range(NCORES)))
    total = np.float64(0.0)
    for r in res.results:
        total += np.asarray(r["out"], np.float64).sum()
    return np.float32(total / (B * P))


# revision 19
# speedup vs baseline: 1.2443x; 1.0352x over previous
"""Trainium2 Bass kernel for nn_ADDLossSoftEncode (Davenport q-method ADD loss).

Data parallel over batch: B=512 sharded as 64 samples per core across 8 cores.
Per core:
  stage A: e = exp(s - max), Gram A_ij = sum_k e_k q_ki q_kj via fused
           tensor_tensor_reduce ops (layout: partition = khalf*64 + sample).
  stage B: power iteration on 4x4 A per sample -> top eigenvector q_pred;
           rotation matrices R(q_pred), R(gt); G = dR dR^T (3x3).
  stage C: per-point squared norm p^T G p via Horner form, sqrt+accumulate
           on ACT -> per-partition partial sums.
Host: sum partials over cores/partitions, divide by B*P.
"""

import sys
from contextlib import ExitStack

import numpy as np

sys.path.insert(0, "/opt/trn_rl_repo")

import concourse.bass as bass
import concourse.tile as tile
from concourse import bacc
from concourse import mybir

F32 = mybir.dt.float32
AX = mybir.AxisListType
OP = mybir.AluOpType
ACT = mybir.ActivationFunctionType

B, K, P = 512, 8192, 4096
NCORES = 8
S = B // NCORES          # 64 samples per core
KH = K // 2              # 4096 per k-half
PH = P // 2              # 2048 per point-half
KC = 1024                # k-chunk (per half)
PC = 512                 # point chunk (per half)
NKC = KH // KC           # 4
NPC = PH // PC           # 2

# unique Gram entries: 9 computed pairs + A33 via trace identity
PAIRS = [(0, 0), (0, 1), (0, 2), (0, 3), (1, 1), (1, 2), (1, 3), (2, 2), (2, 3)]
UIDX = {p: n for n, p in enumerate(PAIRS)}
UIDX[(3, 3)] = 9
NSQ = 13  # matrix squarings (A -> A^(2^NSQ))


def _emit(ctx, tc, sep, ori, gt, pt, out):
    nc = tc.nc
    pool_big = ctx.enter_context(tc.tile_pool(name="big", bufs=2))
    pool_q = ctx.enter_context(tc.tile_pool(name="q", bufs=2))
    pool_u = ctx.enter_context(tc.tile_pool(name="u", bufs=2))
    pool_scr = ctx.enter_context(tc.tile_pool(name="scr", bufs=2))
    pool_st = ctx.enter_context(tc.tile_pool(name="st", bufs=1))
    pool_tiny = ctx.enter_context(tc.tile_pool(name="tiny", bufs=1))
    pool_pt = ctx.enter_context(tc.tile_pool(name="pt", bufs=2))
    pool_c = ctx.enter_context(tc.tile_pool(name="c", bufs=2))

    # ---------------- stage A: softmax weights + Gram matrix ----------------
    s_sb = pool_st.tile([128, KH], F32)
    nc.sync.dma_start(s_sb[0:S, :], sep[:, 0:KH])
    nc.sync.dma_start(s_sb[S:128, :], sep[:, KH:K])

    m2 = pool_tiny.tile([128, 1], F32)
    nc.vector.tensor_reduce(m2[:], s_sb[:], axis=AX.X, op=OP.max)
    negm = pool_tiny.tile([128, 1], F32)
    mup = pool_tiny.tile([64, 1], F32)
    # combined max over both k-halves of each sample, negated, broadcast back
    nc.sync.dma_start(mup[:], m2[S:128, :])
    nc.vector.tensor_tensor(negm[0:S, :], m2[0:S, :], mup[:], op=OP.max)
    nc.scalar.mul(negm[0:S, :], negm[0:S, :], -1.0)
    nc.sync.dma_start(negm[S:128, :], negm[0:S, :])

    e_sb = pool_st.tile([128, KH], F32)
    sumE = pool_tiny.tile([128, 1], F32)
    nc.scalar.activation(e_sb[:], s_sb[:], ACT.Exp, bias=negm[:], scale=1.0,
                         accum_out=sumE[:])

    # Gram accumulators per chunk
    aacc = pool_tiny.tile([128, 9 * NKC], F32)
    for c in range(NKC):
        qc = pool_q.tile([128, KC * 4], F32)
        nc.sync.dma_start(qc[0:S, :],
                            ori[:, c * KC:(c + 1) * KC, :].rearrange("s k q -> s (k q)"))
        nc.sync.dma_start(qc[S:128, :],
                            ori[:, KH + c * KC:KH + (c + 1) * KC, :].rearrange("s k q -> s (k q)"))
        ec = e_sb[:, c * KC:(c + 1) * KC]
        us = []
        for i in range(3):
            ui = pool_u.tile([128, KC], F32, tag=f"u{i}")
            nc.gpsimd.tensor_tensor(ui[:], ec, qc[:, i::4], op=OP.mult)
            us.append(ui)
        for n, (i, j) in enumerate(PAIRS):
            scr = pool_scr.tile([128, KC], F32)
            eng = nc.gpsimd if n % 5 == 4 else nc.vector
            eng.tensor_tensor(scr[:], us[i][:], qc[:, j::4], op=OP.mult)
            scr2 = pool_scr.tile([128, KC], F32, tag="scr2")
            nc.scalar.activation(scr2[:], scr[:], ACT.Identity,
                                 accum_out=aacc[:, n * NKC + c:n * NKC + c + 1])

    # sum chunks -> [128, 9]; then halves; then A33 from trace identity
    a128 = pool_tiny.tile([128, 9], F32)
    nc.vector.tensor_reduce(a128[:], aacc[:].rearrange("p (n c) -> p n c", c=NKC), axis=AX.X,
                            op=OP.add)
    a64 = pool_tiny.tile([64, 10], F32)
    aup = pool_tiny.tile([64, 9], F32)
    nc.sync.dma_start(aup[:], a128[S:128, :])
    nc.vector.tensor_tensor(a64[:, 0:9], a128[0:S, :], aup[:], op=OP.add)
    sE = pool_tiny.tile([64, 1], F32)
    sup = pool_tiny.tile([64, 1], F32)
    nc.sync.dma_start(sup[:], sumE[S:128, :])
    nc.vector.tensor_tensor(sE[:], sumE[0:S, :], sup[:], op=OP.add)
    nc.vector.tensor_tensor(a64[:, 9:10], sE[:], a64[:, UIDX[(0, 0)]:UIDX[(0, 0)] + 1],
                            op=OP.subtract)
    nc.vector.tensor_tensor(a64[:, 9:10], a64[:, 9:10],
                            a64[:, UIDX[(1, 1)]:UIDX[(1, 1)] + 1], op=OP.subtract)
    nc.vector.tensor_tensor(a64[:, 9:10], a64[:, 9:10],
                            a64[:, UIDX[(2, 2)]:UIDX[(2, 2)] + 1], op=OP.subtract)

    # full 4x4 matrix [64, 16] row-major (i*4+j)
    amat = pool_tiny.tile([64, 16], F32)
    for i in range(4):
        for j in range(4):
            u = UIDX[(min(i, j), max(i, j))]
            nc.vector.tensor_copy(amat[:, i * 4 + j:i * 4 + j + 1], a64[:, u:u + 1])

    # ---------------- stage B: matrix squaring (A -> A^8192) ----------------
    # eigengap is ~1.002-1.03, so plain power iteration cannot converge;
    # 13 trace-normalized squarings give (l1/l2)^8192 separation.
    a_cur = amat
    trv = pool_tiny.tile([64, 1], F32)
    tri = pool_tiny.tile([64, 1], F32)
    t64 = pool_tiny.tile([64, 64], F32)
    for m in range(NSQ):
        a_new = pool_tiny.tile([64, 16], F32, tag=f"asq{m % 2}")
        # A2[p,i,k] = sum_j A[p,i,j] * A[p,j,k] in two DVE ops (4D APs)
        in0 = a_cur[:].rearrange("p (i j) -> p i j", j=4).unsqueeze(2) \
                      .broadcast_to([64, 4, 4, 4])
        in1 = a_cur[:].rearrange("p (j k) -> p k j", k=4).unsqueeze(1) \
                      .broadcast_to([64, 4, 4, 4])
        nc.vector.tensor_tensor(
            t64[:].rearrange("p (i k j) -> p i k j", k=4, j=4), in0, in1,
            op=OP.mult)
        nc.vector.tensor_reduce(
            a_new[:].rearrange("p (i k) -> p i k", k=4),
            t64[:].rearrange("p (ik j) -> p ik j", j=4), axis=AX.X, op=OP.add)
        if m % 2 == 1:
            nc.vector.tensor_reduce(trv[:], a_new[:, 0::5], axis=AX.X, op=OP.add)
            nc.vector.reciprocal(tri[:], trv[:])
            nc.vector.tensor_scalar(a_new[:], a_new[:], tri[:], None, op0=OP.mult)
        a_cur = a_new

    # v = A^N @ x0 for fixed x0
    v = pool_tiny.tile([64, 4], F32)
    wt = pool_tiny.tile([64, 4], F32)
    x0 = [1.0, 0.61, 0.37, 0.22]
    for j in range(4):
        colj = a_cur[:, j::4]
        if j == 0:
            nc.scalar.mul(v[:], colj, x0[j])
        else:
            nc.scalar.mul(wt[:], colj, x0[j])
            nc.vector.tensor_tensor(v[:], v[:], wt[:], op=OP.add)

    gtt = pool_tiny.tile([64, 4], F32)
    nc.sync.dma_start(gtt[:], gt[:, :])

    def quat_to_R(q, Rt):
        # outer products o[a*4+b] = q_a*q_b  via 4 tensor_scalar ops
        o = pool_tiny.tile([64, 16], F32, tag="outer")
        for a in range(4):
            nc.vector.tensor_scalar(o[:, a * 4:a * 4 + 4], q[:], q[:, a:a + 1],
                                    None, op0=OP.mult)
        nsq = pool_tiny.tile([64, 1], F32, tag="nsq")
        nc.vector.tensor_tensor(nsq[:], o[:, 0:1], o[:, 5:6], op=OP.add)
        nc.vector.tensor_tensor(nsq[:], nsq[:], o[:, 10:11], op=OP.add)
        nc.vector.tensor_tensor(nsq[:], nsq[:], o[:, 15:16], op=OP.add)
        ts2 = pool_tiny.tile([64, 1], F32, tag="ts2")
        nc.vector.reciprocal(ts2[:], nsq[:])
        nc.scalar.mul(ts2[:], ts2[:], 2.0)
        nts = pool_tiny.tile([64, 1], F32, tag="nts")
        nc.scalar.mul(nts[:], ts2[:], -1.0)
        t = pool_tiny.tile([64, 1], F32, tag="qrt")
        # (r,i,j,k) = (0,1,2,3); o[a,b] at col a*4+b
        def oc(a, b):
            return o[:, a * 4 + b:a * 4 + b + 1]
        # diag entries: R_dd = 1 - ts*(x+y)
        for d, (u1, u2) in enumerate([((2, 2), (3, 3)), ((1, 1), (3, 3)),
                                      ((1, 1), (2, 2))]):
            nc.vector.tensor_tensor(t[:], oc(*u1), oc(*u2), op=OP.add)
            nc.vector.tensor_scalar(Rt[:, d * 3 + d:d * 3 + d + 1], t[:], nts[:],
                                    1.0, op0=OP.mult, op1=OP.add)
        # off-diag: R_mn = ts*(o1 +/- o2)
        offd = [(0, 1, (1, 2), (3, 0), OP.subtract), (0, 2, (1, 3), (2, 0), OP.add),
                (1, 0, (1, 2), (3, 0), OP.add), (1, 2, (2, 3), (1, 0), OP.subtract),
                (2, 0, (1, 3), (2, 0), OP.subtract), (2, 1, (2, 3), (1, 0), OP.add)]
        for m, n, p1, p2, op in offd:
            nc.vector.tensor_tensor(t[:], oc(*p1), oc(*p2), op=op)
            nc.vector.tensor_scalar(Rt[:, m * 3 + n:m * 3 + n + 1], t[:], ts2[:],
                                    None, op0=OP.mult)

    Rp = pool_tiny.tile([64, 9], F32)
    Rg = pool_tiny.tile([64, 9], F32)
    quat_to_R(v, Rp)
    quat_to_R(gtt, Rg)
    dR = pool_tiny.tile([64, 9], F32)
    nc.vector.tensor_tensor(dR[:], Rp[:], Rg[:], op=OP.subtract)

    # G (3x3 sym of dR dR^T): cols [G00,G11,G22,2G01,2G02,2G12] on 128 partitions
    gp = pool_tiny.tile([128, 6], F32)
    gscr = pool_tiny.tile([64, 3], F32)
    gpairs = [(0, 0, 1.0), (1, 1, 1.0), (2, 2, 1.0), (0, 1, 2.0), (0, 2, 2.0),
              (1, 2, 2.0)]
    for n, (a, b, sc) in enumerate(gpairs):
        nc.vector.tensor_tensor(gscr[:], dR[:, 3 * a:3 * a + 3],
                                dR[:, 3 * b:3 * b + 3], op=OP.mult)
        nc.vector.tensor_reduce(gp[0:S, n:n + 1], gscr[:], axis=AX.X, op=OP.add)
        if sc != 1.0:
            nc.scalar.mul(gp[0:S, n:n + 1], gp[0:S, n:n + 1], sc)
    nc.sync.dma_start(gp[S:128, :], gp[0:S, :])

    # ---------------- stage C: per-point norms ----------------
    pn = pool_tiny.tile([128, NPC], F32)
    sqb = pool_tiny.tile([128, 1], F32)
    nc.vector.memset(sqb[:], 1e-5)
    for c in range(NPC):
        ptc = pool_pt.tile([128, PC * 3], F32)
        nc.sync.dma_start(ptc[0:S, :],
                            pt[:, c * PC:(c + 1) * PC, :].rearrange("s k q -> s (k q)"))
        nc.sync.dma_start(ptc[S:128, :],
                            pt[:, PH + c * PC:PH + (c + 1) * PC, :].rearrange("s k q -> s (k q)"))
        x, y, z = ptc[:, 0::3], ptc[:, 1::3], ptc[:, 2::3]

        def g(n):
            return gp[:, n:n + 1]
        # features xx,yy,zz,xy,xz,yz — independent of eigen chain, overlap it
        feats = []
        fsrc = [(x, x), (y, y), (z, z), (x, y), (x, z), (y, z)]
        for n, (fa, fb) in enumerate(fsrc):
            f = pool_c.tile([128, PC], F32, tag=f"f{n}")
            eng = nc.gpsimd if n in (2, 5) else nc.vector
            eng.tensor_tensor(f[:], fa, fb, op=OP.mult)
            feats.append(f)
        # S = sum_m g_m * f_m : scaled on ACT, summed on DVE/gpsimd
        sg = []
        for n in range(6):
            fg = pool_c.tile([128, PC], F32, tag=f"fg{n}")
            nc.scalar.activation(fg[:], feats[n][:], ACT.Identity, scale=g(n))
            sg.append(fg)
        A1 = pool_c.tile([128, PC], F32, tag="A1")
        B1 = pool_c.tile([128, PC], F32, tag="B1")
        nc.vector.tensor_tensor(A1[:], sg[0][:], sg[1][:], op=OP.add)
        nc.gpsimd.tensor_tensor(B1[:], sg[2][:], sg[3][:], op=OP.add)
        nc.vector.tensor_tensor(A1[:], A1[:], sg[4][:], op=OP.add)
        nc.vector.tensor_tensor(B1[:], B1[:], sg[5][:], op=OP.add)
        nc.vector.tensor_tensor(A1[:], A1[:], B1[:], op=OP.add)
        sq = pool_c.tile([128, PC], F32, tag="sq")
        nc.scalar.activation(sq[:], A1[:], ACT.Sqrt, bias=sqb[:], scale=1.0,
                             accum_out=pn[:, c:c + 1])

    partial = pool_tiny.tile([128, 1], F32)
    nc.vector.tensor_reduce(partial[:], pn[:], axis=AX.X, op=OP.add)
    nc.sync.dma_start(out[:, :], partial[:])


_NC_CACHE = {}


def _build():
    if "nc" in _NC_CACHE:
        return _NC_CACHE["nc"]
    nc = bacc.Bacc("TRN2", target_bir_lowering=False, debug=False, enable_asserts=True, num_devices=NCORES)
    sep = nc.declare_dram_parameter("softEncodePred", [S, K], F32, isOutput=False)
    ori = nc.declare_dram_parameter("oriHistogramMap", [S, K, 4], F32, isOutput=False)
    gt = nc.declare_dram_parameter("gt", [S, 4], F32, isOutput=False)
    pt = nc.declare_dram_parameter("point", [S, P, 3], F32, isOutput=False)
    out = nc.declare_dram_parameter("out", [128, 1], F32, isOutput=True)
    with tile.TileContext(nc) as tc:
        with ExitStack() as ctx:
            _emit(ctx, tc, sep.ap(), ori.ap(), gt.ap(), pt.ap(), out.ap())
    nc.finalize()
    _NC_CACHE["nc"] = nc
    return nc


def kernel(softEncodePred, oriHistogramMap, gt, point):
    nc = _build()
    in_maps = []
    for c in range(NCORES):
        sl = slice(c * S, (c + 1) * S)
        in_maps.append({
            "softEncodePred": np.ascontiguousarray(softEncodePred[sl], np.float32),
            "oriHistogramMap": np.ascontiguousarray(oriHistogramMap[sl], np.float32),
            "gt": np.ascontiguousarray(gt[sl], np.float32),
            "point": np.ascontiguousarray(point[sl], np.float32),
        })
    from concourse.bass_utils import run_bass_kernel_spmd
    res = run_bass_kernel_spmd(nc, in_maps, core_ids=list(range(NCORES)))
    total = np.float64(0.0)
    for r in res.results:
        total += np.asarray(r["out"], np.float64).sum()
    return np.float32(total / (B * P))
